# revision 42
# baseline (speedup 1.0000x reference)
"""Trainium2 Bass kernel for GQA multi-head attention (B=2,S=2048,HID=2048,H=32,KVH=8,D=64).

Sharding: 8 cores = 2 (batch) x 4 (kv-head groups). Each core handles one batch
element and 2 kv heads (= 8 q heads), computes its partial o_proj output
(contracting only its 512 attention features), host sums 4 partials per batch.

Device layouts (host pre-marshalled):
  hsT   [HID, S]   transposed hidden states for this core's batch
  cosT2 [128, S]   cos.T replicated for the 2 heads packed per partition-block
  sinT2 [128, S]   sin.T replicated
  rotm  [128, 128] transposed rotate_half matrix (R.T with R q = rotate_half(q))
  wqT   [HID, 512] Wq rows for this core's q heads (perm order), transposed
  wkT   [HID, 128] Wk rows for 2 kv heads, transposed
  wvT   [HID, 128]
  woT   [512, HID] Wo columns for this core's features (perm order), transposed
  out   [S, HID]   partial output (sum over cores with same batch on host)

Head perm order per core: local q heads [0,4,1,5,2,6,3,7] so each 128-partition
q tile j holds (head needing kv0 at partitions 0:64, head needing kv1 at 64:128),
aligning with kT/vT partition halves for row-packed matmuls.

Softmax: scores are O(10) so exp without max-subtraction is safe in fp32.
Attn@V uses zero-padded 128-col lhsT tiles ([vA|0], [0|vB]) accumulating into
one PSUM tile so head B lands on partitions 64:128 with no partition-move
DMAs. Row sums come from ones-lhsT matmuls into a separate 2-bank PSUM tile;
1/rowsum = exp(-ln r) on the scalar engine (Ln/Exp share one table set with
the phase-B Exps), then a PE outer-product broadcast + one multiply.

Output is written as bf16 [S, HID] partials in contiguous full-row DMAs; the
host upcasts and sums 4 partials per batch. (Strided f32 partial stores and
per-iteration SBUF-to-SBUF DMAs / 1-partition DVE reciprocals dominated the
benchmark's per-execution cost in the previous version.)
"""

import sys

if "/opt/trn_rl_repo" not in sys.path:
    sys.path.insert(0, "/opt/trn_rl_repo")

import numpy as np

B, S, HID = 2, 2048, 2048
H, KVH, D = 32, 8, 64
NCORES = 8

PERM_LOCAL = [0, 4, 1, 5, 2, 6, 3, 7]

_NC_CACHE = {}


def _build_nc():
    import concourse.bass as bass
    import concourse.mybir as mybir
    from concourse import bacc
    from concourse.tile import TileContext
    from concourse.masks import make_identity
    from contextlib import ExitStack

    f32 = mybir.dt.float32
    f32r = mybir.dt.float32r
    bf16 = mybir.dt.bfloat16
    Exp = mybir.ActivationFunctionType.Exp
    Ln = mybir.ActivationFunctionType.Ln
    mult = mybir.AluOpType.mult
    add = mybir.AluOpType.add

    nc = bacc.Bacc(None, target_bir_lowering=False)

    hsT = nc.declare_dram_parameter("hsT", [HID, S], f32r, isOutput=False)
    cosT2 = nc.declare_dram_parameter("cosT2", [128, S], f32, isOutput=False)
    sinT2 = nc.declare_dram_parameter("sinT2", [128, S], f32, isOutput=False)
    rotm = nc.declare_dram_parameter("rotm", [128, 128], f32r, isOutput=False)
    wqT = nc.declare_dram_parameter("wqT", [HID, 512], f32r, isOutput=False)
    wkT = nc.declare_dram_parameter("wkT", [HID, 128], f32r, isOutput=False)
    wvT = nc.declare_dram_parameter("wvT", [HID, 128], f32r, isOutput=False)
    woT = nc.declare_dram_parameter("woT", [512, HID], f32r, isOutput=False)
    out = nc.declare_dram_parameter("out", [S, HID], bf16, isOutput=True)

    KT = HID // 128  # 16 contraction k-tiles for projections
    SCA = 256        # phase-A s-chunk width
    NSCA = S // SCA  # 8
    TT = S // 128    # 16 t-tiles
    SCB = 512        # phase-B s-chunk width
    NSCB = S // SCB  # 4

    with TileContext(nc) as tc, ExitStack() as ctx:
        # ---------------- pools ----------------
        persist = ctx.enter_context(tc.tile_pool(name="persist", bufs=1))
        qT_sb = persist.tile([128, 4, S], f32r)       # q' transposed, 4 e-tiles
        kT_sb = persist.tile([128, S], f32r)          # k' transposed (2 kv heads)
        v_sbA = persist.tile([128, TT, 128], f32r)    # [vA | 0] padded lhsT
        v_sbB = persist.tile([128, TT, 128], f32r)    # [0 | vB] padded lhsT
        ones_sb = persist.tile([128, 256], f32r)
        ones_bf = persist.tile([128, 256], bf16)
        rot_sb = persist.tile([128, 128], f32r)


        # PSUM: scores pool 3x2banks + small pool 2x1bank = 8 banks
        sp = ctx.enter_context(tc.tile_pool(name="sp", bufs=2, space="PSUM"))
        op = ctx.enter_context(tc.tile_pool(name="op", bufs=2, space="PSUM"))

        # phase-B P^T staging
        ptp = ctx.enter_context(tc.tile_pool(name="ptp", bufs=2))
        rrp = ctx.enter_context(tc.tile_pool(name="rrp", bufs=2))

        onesf = persist.tile([128, 256], f32)
        zf = persist.tile([128, 64], f32)
        nc.vector.memset(zf, 0.0)
        nc.vector.memset(onesf, 1.0)
        # masks for the two normalize outer-products (any partition row works):
        # cols 0:128 -> head A mask (1 on out partitions 0:64), cols 128:256 ->
        # head B mask (1 on 64:128)
        nc.vector.memset(onesf[:, 64:192], 0.0)
        nc.vector.tensor_copy(ones_sb, onesf[:, 0:256])
        nc.vector.tensor_copy(ones_bf, onesf[:, 0:256])
        nc.sync.dma_start(out=rot_sb, in_=rotm[:, :])

        # ---------------- phase A: projections + RoPE + v transpose -------------
        actx = ExitStack()
        pcs = actx.enter_context(tc.tile_pool(name="pcs", bufs=1))
        cos_sb = pcs.tile([128, S], f32)
        sin_sb = pcs.tile([128, S], f32)
        nc.sync.dma_start(out=cos_sb, in_=cosT2[:, :])
        nc.sync.dma_start(out=sin_sb, in_=sinT2[:, :])

        hsp = actx.enter_context(tc.tile_pool(name="hsp", bufs=2))
        ropep = actx.enter_context(tc.tile_pool(name="ropep", bufs=2))

        a1ctx = ExitStack()
        pkv = a1ctx.enter_context(tc.tile_pool(name="pkv", bufs=1))
        wk_sb = pkv.tile([128, KT, 128], f32r)
        wv_sb = pkv.tile([128, KT, 128], f32r)
        ident = pkv.tile([128, 128], f32)
        nc.sync.dma_start(out=wk_sb, in_=wkT.rearrange("(t p) e -> p t e", p=128))
        nc.sync.dma_start(out=wv_sb, in_=wvT.rearrange("(t p) e -> p t e", p=128))
        make_identity(nc, ident)
        vstg = a1ctx.enter_context(tc.tile_pool(name="vstg", bufs=2))

        def rope_drain(ps, rot_ps, dst, sl):
            """ps: [128, SCA] psum q/k tile; rot_ps: [128, SCA] spare psum slot.
            Computes dst[:, sl] = ps*cos + (R@ps)*sin."""
            qsb = ropep.tile([128, SCA], f32r, name="qsb", tag="qsb")
            nc.scalar.copy(qsb, ps)
            nc.tensor.matmul(rot_ps, rot_sb, qsb, start=True, stop=True)
            nc.vector.tensor_tensor(out=dst[:, sl], in0=qsb, in1=cos_sb[:, sl],
                                    op=mult)
            shs = ropep.tile([128, SCA], f32, name="shs", tag="shs")
            nc.vector.tensor_tensor(out=shs, in0=rot_ps, in1=sin_sb[:, sl], op=mult)
            nc.vector.tensor_tensor(out=dst[:, sl], in0=dst[:, sl], in1=shs, op=add)

        # Single pass: K, V, and all 4 Q e-tiles from one hs chunk load
        pq = a1ctx.enter_context(tc.tile_pool(name="pq", bufs=1))
        wq_sb = pq.tile([128, KT, 512], f32r)
        nc.sync.dma_start(out=wq_sb, in_=wqT.rearrange("(t p) e -> p t e", p=128))

        for sc in range(NSCA):
            sl = slice(sc * SCA, (sc + 1) * SCA)
            hs_sb = hsp.tile([128, KT, SCA], f32r, name="hs_sb")
            nc.sync.dma_start(
                out=hs_sb, in_=hsT[:, sl].rearrange("(t p) s -> p t s", p=128))
            kv_ps = sp.tile([128, 2, 512], f32, name="kv_ps", tag="sp")
            for ki in range(KT):
                nc.tensor.matmul(kv_ps[:, 0, 0:SCA], wk_sb[:, ki, :],
                                 hs_sb[:, ki, :], start=ki == 0, stop=ki == KT - 1)
            for ki in range(KT):
                nc.tensor.matmul(kv_ps[:, 0, SCA : 2 * SCA], wv_sb[:, ki, :],
                                 hs_sb[:, ki, :], start=ki == 0, stop=ki == KT - 1)
            # k: RoPE into kT_sb (rot output borrows slot 1 of same psum tile)
            rope_drain(kv_ps[:, 0, 0:SCA], kv_ps[:, 1, 0:SCA], kT_sb, sl)
            # v: stage, transpose 128-blocks into padded v_sbA/v_sbB layout
            vt_sb = vstg.tile([128, SCA], f32, name="vt_sb")
            nc.scalar.copy(vt_sb, kv_ps[:, 0, SCA : 2 * SCA])
            for i in range(SCA // 128):
                tt = (sc * SCA) // 128 + i
                tps = op.tile([128, 512], f32, name="tps", tag="o")
                nc.tensor.transpose(tps[:, 0:128], vt_sb[:, i * 128 : (i + 1) * 128],
                                    ident)
                nc.vector.tensor_copy(v_sbA[:, tt, 0:64], tps[:, 0:64])
                nc.vector.tensor_copy(v_sbA[:, tt, 64:128], zf)
                nc.vector.tensor_copy(v_sbB[:, tt, 0:64], zf)
                nc.vector.tensor_copy(v_sbB[:, tt, 64:128], tps[:, 64:128])
            for jj in range(2):  # two psum tiles, 2 e-tiles each
                q_ps = sp.tile([128, 2, 512], f32, name="q_ps", tag="sp")
                for half in range(2):
                    j = jj * 2 + half
                    dst_sl = slice(half * SCA, (half + 1) * SCA)
                    for ki in range(KT):
                        nc.tensor.matmul(
                            q_ps[:, 0, dst_sl],
                            wq_sb[:, ki, j * 128 : (j + 1) * 128],
                            hs_sb[:, ki, :], start=ki == 0, stop=ki == KT - 1)
                    rope_drain(q_ps[:, 0, dst_sl], q_ps[:, 1, dst_sl],
                               qT_sb[:, j, :], sl)

        a1ctx.close()
        actx.close()  # release phase-A pools so phase-B/C can reuse SBUF

        # ---------------- phase B + C interleaved, per s-chunk -----------------
        # Padded-lhsT attn@V: [vA|0] then [0|vB] accumulate into one PSUM tile
        # so head B lands on partitions 64:128 with no partition moves. Rowsums
        # via ones-lhsT matmuls (A bank 0, B bank 1, replicated on partitions
        # 0:32); 1/r = exp(-ln r) on ACT (same table set as the Exps). o_proj for
        # each s-chunk is emitted right after its 4 head-pairs finish, giving
        # the tensor engine dense work while ACT runs the next chunk's exps.
        pc = ctx.enter_context(tc.tile_pool(name="pc", bufs=1))
        oT_sb = pc.tile([128, 4, S], f32r)            # normalized attn out ^T
        wo_sb = pc.tile([128, 4, HID], f32r)
        nc.sync.dma_start(out=wo_sb, in_=woT.rearrange("(t p) h -> p t h", p=128))
        ostg = ctx.enter_context(tc.tile_pool(name="ostg", bufs=3))
        nrm = ctx.enter_context(tc.tile_pool(name="nrm", bufs=2))

        def norm_and_oproj(psc, prsc):
            """Normalize chunk psc's oT (batched 1/r = exp(-ln r); ln scratch
            must be f32 — abs error of ln becomes rel error after exp) and run
            its o_proj. Deferred to after the NEXT chunk's first head-pair so
            the PE has dense score/attnV work while ACT runs this chain."""
            psl = slice(psc * SCB, (psc + 1) * SCB)
            lnb = nrm.tile([128, 4, 2, 512], f32, name="lnb", bufs=1)
            rvb = nrm.tile([128, 4, 2, 512], bf16, name="rvb", bufs=1)
            nc.scalar.activation(lnb[0:1, :, :, :], prsc[0:1, :, :, :], Ln)
            nc.scalar.activation(rvb[0:1, :, :, :], lnb[0:1, :, :, :], Exp,
                                 scale=-1.0)
            for j in range(4):
                bc = sp.tile([128, 2, 512], f32, name="bc", tag="sp")
                nc.tensor.matmul(bc[:, 0, :], ones_bf[0:1, 0:128],
                                 rvb[0:1, j, 0, :], start=True, stop=False)
                nc.tensor.matmul(bc[:, 0, :], ones_bf[0:1, 128:256],
                                 rvb[0:1, j, 1, :], start=False, stop=True)
                nc.vector.tensor_tensor(out=oT_sb[:, j, psl],
                                        in0=oT_sb[:, j, psl],
                                        in1=bc[:, 0, :], op=mult)
            for k in range(SCB // 128):
                st = psc * (SCB // 128) + k
                ssl = slice(st * 128, (st + 1) * 128)
                og = ostg.tile([128, HID], bf16, name="og")
                for hc in range(HID // 512):
                    hsl = slice(hc * 512, (hc + 1) * 512)
                    ops = op.tile([128, 512], f32, name="ops", tag="o")
                    for et in range(4):
                        nc.tensor.matmul(ops, oT_sb[:, et, ssl],
                                         wo_sb[:, et, hsl],
                                         start=et == 0, stop=et == 3)
                    nc.vector.tensor_copy(og[:, hsl], ops)
                nc.sync.dma_start(out=out[ssl, :], in_=og)

        pending = None
        for sc in range(NSCB):
            sl = slice(sc * SCB, (sc + 1) * SCB)
            rsc = nrm.tile([128, 4, 2, 512], bf16, name="rsc", bufs=2)
            for j in range(4):
                qA = qT_sb[0:64, j, sl]
                qB = qT_sb[64:128, j, sl]
                oAB = op.tile([128, 512], f32, name="oAB", tag="o")
                rs_ps = op.tile([128, 2, 512], f32, name="rs_ps", tag="o")
                for g in range(TT // 2):
                    sA = sp.tile([128, 2, 512], f32, name="sA", tag="sp")
                    sB = sp.tile([128, 2, 512], f32, name="sB", tag="sp")
                    for i in range(2):
                        tt = 2 * g + i
                        ksl = slice(tt * 128, (tt + 1) * 128)
                        nc.tensor.matmul(sA[:, i, :], kT_sb[0:64, ksl], qA,
                                         start=True, stop=True,
                                         tile_position=(0, 0))
                        nc.tensor.matmul(sB[:, i, :], kT_sb[64:128, ksl], qB,
                                         start=True, stop=True,
                                         tile_position=(64, 0))
                    pA = ptp.tile([128, 2, 512], f32r, name="pA", tag="pt")
                    pB = ptp.tile([128, 2, 512], f32r, name="pB", tag="pt")
                    nc.scalar.activation(pA, sA, Exp, scale=0.125)
                    nc.scalar.activation(pB, sB, Exp, scale=0.125)
                    for i in range(2):
                        tt = 2 * g + i
                        st = tt == 0
                        sp_ = tt == TT - 1
                        nc.tensor.matmul(oAB, v_sbA[:, tt, :],
                                         pA[:, i, :], start=st, stop=False)
                        nc.tensor.matmul(oAB, v_sbB[:, tt, :],
                                         pB[:, i, :], start=False, stop=sp_)
                        nc.tensor.matmul(rs_ps[0:32, 0, :], ones_sb[:, 0:32],
                                         pA[:, i, :], start=st, stop=sp_)
                        nc.tensor.matmul(rs_ps[0:32, 1, :], ones_sb[:, 0:32],
                                         pB[:, i, :], start=st, stop=sp_)
                # stash this j's rowsum rows; normalize is batched per sc so
                # the ACT table set only switches twice per chunk, not per j
                nc.vector.tensor_copy(rsc[0:1, j, :, :], rs_ps[0:1, :, :])
                nc.vector.tensor_copy(oT_sb[:, j, sl], oAB[:, :])
                if j == 0 and pending is not None:
                    norm_and_oproj(*pending)
            pending = (sc, rsc)
        norm_and_oproj(*pending)

    nc.finalize()
    return nc


def _get_nc():
    if "nc" not in _NC_CACHE:
        _NC_CACHE["nc"] = _build_nc()
    return _NC_CACHE["nc"]


def _rot_matrix():
    # R @ q = rotate_half(q) per 64-block: R[i, i+32] = -1 (i%64<32),
    # R[i, i-32] = +1 (i%64>=32). Device needs lhsT = R.T.
    R = np.zeros((128, 128), dtype=np.float32)
    for blk in (0, 64):
        for i in range(32):
            R[blk + i, blk + i + 32] = -1.0
            R[blk + 32 + i, blk + i] = 1.0
    return np.ascontiguousarray(R.T)


def _marshal(inputs):
    hs = np.asarray(inputs["hidden_states"], dtype=np.float32)
    cos = np.asarray(inputs["cos"], dtype=np.float32)
    sin = np.asarray(inputs["sin"], dtype=np.float32)
    Wq = np.asarray(inputs["Wq"], dtype=np.float32)
    Wk = np.asarray(inputs["Wk"], dtype=np.float32)
    Wv = np.asarray(inputs["Wv"], dtype=np.float32)
    Wo = np.asarray(inputs["Wo"], dtype=np.float32)

    c = np.ascontiguousarray
    rotm = _rot_matrix()
    in_maps = []
    for core in range(NCORES):
        b, kg = divmod(core, 4)
        gheads = [kg * 8 + l for l in PERM_LOCAL]
        kvh = [2 * kg, 2 * kg + 1]
        wqT = c(Wq.reshape(H, D, HID)[gheads].reshape(512, HID).T)
        wkT = c(Wk.reshape(KVH, D, HID)[kvh].reshape(128, HID).T)
        wvT = c(Wv.reshape(KVH, D, HID)[kvh].reshape(128, HID).T)
        woT = c(Wo.T.reshape(H, D, HID)[gheads].reshape(512, HID))
        hsT = c(hs[b].T)
        cosT = cos[b].T  # [64, S]
        sinT = sin[b].T
        cosT2 = c(np.concatenate([cosT, cosT], axis=0))
        sinT2 = c(np.concatenate([sinT, sinT], axis=0))
        in_maps.append({
            "hsT": hsT, "cosT2": cosT2, "sinT2": sinT2, "rotm": rotm,
            "wqT": wqT, "wkT": wkT, "wvT": wvT, "woT": woT,
        })
    return in_maps


def run(inputs, trace=False, trace_cores=None):
    from concourse.bass_utils import run_bass_kernel_spmd

    nc = _get_nc()
    in_maps = _marshal(inputs)
    res = run_bass_kernel_spmd(
        nc, in_maps, core_ids=list(range(NCORES)), trace=trace,
        trace_cores=trace_cores)
    outs = [np.asarray(res.results[i]["out"]).astype(np.float32)
            for i in range(NCORES)]
    final = np.zeros((B, S, HID), dtype=np.float32)
    for b in range(B):
        final[b] = outs[4 * b] + outs[4 * b + 1] + outs[4 * b + 2] + outs[4 * b + 3]
    return final, res


def kernel(**inputs):
    out, _ = run(inputs, trace=False)
    return out

